# revision 12
# baseline (speedup 1.0000x reference)
"""CenterLoss kernel for Trainium2 (8 NeuronCores, Bass).

Math: the reference builds the full [B, C] squared-distance matrix, masks it
to one column per row (the label), clips ALL entries to [1e-12, 1e12], sums
and divides by B.  Because the mask keeps exactly one entry per row:

    loss = ( sum_b clip(||x_b - centers[l_b]||^2, 1e-12, 1e12)
             + (B*C - B) * 1e-12 ) / B

so the kernel is a row gather of `centers` plus an elementwise reduction --
no GEMM needed.  For this problem's inputs every per-row distance is
~4096 +- 400, so the clip is mathematically the identity; it is still
applied to the partial row sums before the final reduction.

Sharding: data-parallel over the batch.  Each of the 8 cores receives 512
rows of x (bf16, flat [128, 4*2048]: partition p holds batch rows
4p..4p+3), labels wrapped [128, 4] int32 (lab[p, k] = labels[4p + k]),
and the full centers table (bf16).  Per column block k: one HWDGE DMA for
the x block (alternating the SP/ACT rings so partial sems land early),
one full-row indirect gather of centers, DVE in-place subtract (bf16 2x
mode).  Row sums of squares are split across engines: ACT Square+accum
(blocks 0, 1, first half of 3), GPSIMD mult+reduce (block 2, idle after
descriptor generation), DVE mult+reduce (second half of 3).  The [128, 5]
f32 partial sums are written out (DMA or PE-reduce+register-store path);
the host applies the clip, sums across cores, and adds the clip constant.
bf16 inputs halve HBM traffic; quantization bias on the loss is ~1e-5
relative, far inside the 2e-2 gate.

Hand-placed semaphores (no TileContext) to minimize scheduling overhead.
"""

import numpy as np
import ml_dtypes
from contextlib import ExitStack

import concourse.bacc as bacc
import concourse.bass as bass
import concourse.mybir as mybir
from concourse.bass_utils import run_bass_kernel_spmd

B = 4096
D = 2048
C = 8192
N_CORES = 8
SHARD = B // N_CORES          # 512
P = 128
T = SHARD // P                # 4
HD = D // 2                   # 1024, split point for the last block

BF16 = ml_dtypes.bfloat16

_nc_cache = {}


def _build(final_wait=False, out_reg=False, scratch=16384):
    key = (final_wait, out_reg, scratch)
    if key in _nc_cache:
        return _nc_cache[key]

    nc = bacc.Bacc("TRN2", target_bir_lowering=False, debug=False,
                   dynamic_dma_scratch_size=scratch)
    bf16 = mybir.dt.bfloat16
    f32 = mybir.dt.float32
    x = nc.dram_tensor("x", [P, T * D], bf16, kind="ExternalInput")
    labels = nc.dram_tensor("labels", [P, T], mybir.dt.int32, kind="ExternalInput")
    centers = nc.dram_tensor("centers", [C, D], bf16, kind="ExternalInput")
    out_shape = [1, 1] if out_reg else [P, T + 1]
    out = nc.dram_tensor("out", out_shape, f32, kind="ExternalOutput")

    with ExitStack() as ctx:
        block = ctx.enter_context(nc.Block(no_gpsimd_drain=True))
        lab = ctx.enter_context(nc.sbuf_tensor("lab", [P, T], mybir.dt.int32))
        xall = ctx.enter_context(nc.sbuf_tensor("xall", [P, T * D], bf16))
        gts = [ctx.enter_context(nc.sbuf_tensor(f"gt{t}", [P, D], bf16)) for t in range(T)]
        rowsum = ctx.enter_context(nc.sbuf_tensor("rowsum", [P, T + 1], f32))
        if out_reg:
            ones = ctx.enter_context(nc.sbuf_tensor("ones", [P, 1], f32))
            final = ctx.enter_context(nc.sbuf_tensor("final", [1, 1], f32))
            colsum = ctx.enter_context(nc.psum_tensor("colsum", [1, T + 1], f32))

        s_lab = ctx.enter_context(nc.semaphore("s_lab"))
        s_x = [ctx.enter_context(nc.semaphore(f"s_x{t}")) for t in range(T)]
        s_g = [ctx.enter_context(nc.semaphore(f"s_g{t}")) for t in range(T)]
        s_sub = [ctx.enter_context(nc.semaphore(f"s_sub{t}")) for t in range(T)]
        s_mul = [ctx.enter_context(nc.semaphore(f"s_mul{t}")) for t in range(T)]
        s_acc = ctx.enter_context(nc.semaphore("s_acc"))
        s_out = ctx.enter_context(nc.semaphore("s_out"))
        if out_reg:
            s_ones = ctx.enter_context(nc.semaphore("s_ones"))
            s_clip = ctx.enter_context(nc.semaphore("s_clip"))
            s_mm = ctx.enter_context(nc.semaphore("s_mm"))
            s_red = ctx.enter_context(nc.semaphore("s_red"))

        def blk(t):
            return xall[:, t * D:(t + 1) * D]

        @block.sync
        def _(sync):
            sync.dma_start(out=lab[:, :], in_=labels[:, :]).then_inc(s_lab, 16)
            for t in (0, 2):
                sync.dma_start(out=blk(t), in_=x[:, t * D:(t + 1) * D]
                               ).then_inc(s_x[t], 16)

        @block.gpsimd
        def _(gpsimd):
            if out_reg:
                gpsimd.memset(ones[:, :], 1.0).then_inc(s_ones, 1)
            # col 2 of rowsum holds only block 2's scalar total (partition 0)
            gpsimd.memset(rowsum[:, 2:3], 0.0)
            gpsimd.wait_ge(s_lab, 16)
            for t in range(T):
                gpsimd.indirect_dma_start(
                    out=gts[t][:, :],
                    out_offset=None,
                    in_=centers[:, :],
                    in_offset=bass.IndirectOffsetOnAxis(ap=lab[:, t:t + 1], axis=0),
                ).then_inc(s_g[t], 16)
            # block 2 reduce runs on GPSIMD, which is idle after gather gen;
            # its tensor_reduce can only do a full reduction, which is fine
            # because the clip is the identity for this problem's data
            gpsimd.wait_ge(s_sub[2], 1)
            gpsimd.tensor_tensor(
                out=gts[2][:, :], in0=blk(2), in1=blk(2),
                op=mybir.AluOpType.mult,
            ).then_inc(s_mul[2], 1)
            gpsimd.wait_ge(s_mul[2], 1)
            gpsimd.tensor_reduce(
                out=rowsum[0:1, 2:3], in_=gts[2][:, :],
                axis=mybir.AxisListType.XYZWC, op=mybir.AluOpType.add,
            ).then_inc(s_acc, 1)
            if out_reg:
                gpsimd.wait_ge(s_acc, 5)
                gpsimd.tensor_scalar(
                    out=rowsum[:, :], in0=rowsum[:, :],
                    scalar1=1e-12, scalar2=1e12,
                    op0=mybir.AluOpType.max, op1=mybir.AluOpType.min,
                ).then_inc(s_clip, 1)

        @block.vector
        def _(vector):
            for t in range(T):
                vector.wait_ge(s_x[t], 16)
                vector.wait_ge(s_g[t], 16)
                # in-place: x block <- x - g  (bf16 keeps DVE 2x mode)
                vector.tensor_tensor(
                    out=blk(t), in0=blk(t), in1=gts[t][:, :],
                    op=mybir.AluOpType.subtract,
                ).then_inc(s_sub[t], 1)
            # DVE takes the second half of block 3
            vector.wait_ge(s_sub[3], 1)
            vector.tensor_tensor(
                out=gts[3][:, HD:], in0=blk(3)[:, HD:], in1=blk(3)[:, HD:],
                op=mybir.AluOpType.mult,
            ).then_inc(s_mul[3], 1)
            vector.wait_ge(s_mul[3], 1)
            vector.tensor_reduce(
                out=rowsum[:, 4:5], in_=gts[3][:, HD:],
                axis=mybir.AxisListType.X, op=mybir.AluOpType.add,
            ).then_inc(s_acc, 1)
            if out_reg:
                vector.wait_ge(s_mm, 1)
                vector.tensor_reduce(
                    out=final[:, :], in_=colsum[:1, :],
                    axis=mybir.AxisListType.X, op=mybir.AluOpType.add,
                ).then_inc(s_red, 1)

        @block.scalar
        def _(scalar):
            # x blocks 1, 3 ride the second HWDGE ring (issued before ACT's
            # compute; the auto-inserted ACT_TABLE_LOAD precedes them, which
            # is fine since these are the later-needed blocks)
            for t in (1, 3):
                scalar.dma_start(out=blk(t), in_=x[:, t * D:(t + 1) * D]
                                 ).then_inc(s_x[t], 16)
            for t in (0, 1):
                scalar.wait_ge(s_sub[t], 1)
                scalar.activation(
                    out=blk(t), in_=blk(t),
                    func=mybir.ActivationFunctionType.Square,
                    accum_out=rowsum[:, t:t + 1],
                ).then_inc(s_acc, 1)
            scalar.wait_ge(s_sub[3], 1)
            scalar.activation(
                out=blk(3)[:, :HD], in_=blk(3)[:, :HD],
                func=mybir.ActivationFunctionType.Square,
                accum_out=rowsum[:, 3:4],
            ).then_inc(s_acc, 1)
            if out_reg:
                with scalar.register("gr_out") as gr_out:
                    scalar.wait_ge(s_red, 1)
                    scalar.reg_load(gr_out, final[:1, :1].bitcast(mybir.dt.int32))
                    scalar.reg_save(out[:1, :1].bitcast(mybir.dt.int32), gr_out)
            else:
                scalar.wait_ge(s_acc, 5)
                scalar.dma_start(out=out[:, :], in_=rowsum[:, :]).then_inc(s_out, 16)
                if final_wait:
                    scalar.wait_ge(s_out, 16)

        if out_reg:
            @block.tensor
            def _(tensor):
                tensor.wait_ge(s_clip, 1)
                tensor.wait_ge(s_ones, 1)
                tensor.matmul(
                    colsum[:1, :], ones[:, :], rowsum[:, :], start=True, stop=True,
                ).then_inc(s_mm, 1)

    nc.compile()
    _nc_cache[key] = nc
    return nc


def _make_in_maps(x, labels, centers):
    x = np.asarray(x, dtype=np.float32).astype(BF16)
    centers = np.ascontiguousarray(np.asarray(centers, dtype=np.float32).astype(BF16))
    lab32 = np.asarray(labels).astype(np.int32)
    in_maps = []
    for i in range(N_CORES):
        sl = slice(i * SHARD, (i + 1) * SHARD)
        in_maps.append({
            # partition p holds batch rows 4p..4p+3 of this shard
            "x": np.ascontiguousarray(x[sl]).reshape(P, T * D),
            # lab[p, k] = labels[4p + k], pairing with x column block k
            "labels": np.ascontiguousarray(lab32[sl].reshape(P, T)),
            "centers": centers,
        })
    return in_maps


def _finish(results):
    total = 0.0
    for r in results:
        rs = np.asarray(r["out"], dtype=np.float64)
        if rs.size == 1:
            total += rs[0, 0]
        else:
            # columns 0, 1: row sums for blocks 0, 1; column 3 + column 4:
            # the two halves of block 3; column 2: block 2's scalar total
            # (partition 0; the clip is the identity for it)
            d = np.stack([rs[:, 0], rs[:, 1], rs[:, 3] + rs[:, 4]], axis=1)
            total += np.clip(d, 1e-12, 1e12).sum() + rs[:, 2].sum()
    total += (B * C - B) * 1e-12
    return np.float32(total / B)


def kernel(x, labels, centers):
    nc = _build()
    in_maps = _make_in_maps(x, labels, centers)
    res = run_bass_kernel_spmd(nc, in_maps, core_ids=list(range(N_CORES)))
    return _finish(res.results)


# revision 13
# speedup vs baseline: 1.0923x; 1.0923x over previous
"""CenterLoss kernel for Trainium2 (8 NeuronCores, Bass).

Math: the reference builds the full [B, C] squared-distance matrix, masks it
to one column per row (the label), clips ALL entries to [1e-12, 1e12], sums
and divides by B.  Because the mask keeps exactly one entry per row:

    loss = ( sum_b clip(||x_b - centers[l_b]||^2, 1e-12, 1e12)
             + (B*C - B) * 1e-12 ) / B

so the kernel is a row gather of `centers` plus an elementwise reduction --
no GEMM needed.

Sharding: data-parallel over the batch.  Each of the 8 cores receives 512
rows of x (bf16, flat [128, 4*2048]: partition p holds batch rows
4p..4p+3), labels wrapped [128, 4] int32 (lab[p, k] = labels[4p + k]),
and the full centers table (bf16).

All data movement rides ONE SWDGE ring in exact FIFO need-order
(labels, x0, x1, g0, x2, g1, x3, g2, g3): round-robin across rings would
make every chunk finish near the end of the whole DMA phase, while a
single FIFO ring gives each chunk the full wire in sequence, so compute
pipelines tile-by-tile behind the DMA stream.  Per column block k: DVE
in-place subtract (bf16 keeps the 2x DVE mode).  Row sums of squares:
ACT Square+accumulate for blocks 0..2 and the first half of 3; DVE
mult+reduce for the second half of 3 (splitting the last block shortens
the critical tail).  The [128, 5] f32 partial sums are DMA'd out by the
idle sync engine; the host applies the clip, sums across cores, and adds
the clip constant.  bf16 inputs halve HBM traffic; the quantization bias
on the loss is ~1e-5 relative, far inside the 2e-2 gate.

Hand-placed semaphores (no TileContext) to minimize scheduling overhead.
"""

import numpy as np
import ml_dtypes
from contextlib import ExitStack

import concourse.bacc as bacc
import concourse.bass as bass
import concourse.mybir as mybir
from concourse.bass_utils import run_bass_kernel_spmd

B = 4096
D = 2048
C = 8192
N_CORES = 8
SHARD = B // N_CORES          # 512
P = 128
T = SHARD // P                # 4
HD = D // 2                   # 1024, split point for the last block

BF16 = ml_dtypes.bfloat16

_nc_cache = {}


def _build(final_wait=False, scratch=16384):
    key = (final_wait, scratch)
    if key in _nc_cache:
        return _nc_cache[key]

    nc = bacc.Bacc("TRN2", target_bir_lowering=False, debug=False,
                   dynamic_dma_scratch_size=scratch)
    bf16 = mybir.dt.bfloat16
    f32 = mybir.dt.float32
    x = nc.dram_tensor("x", [P, T * D], bf16, kind="ExternalInput")
    labels = nc.dram_tensor("labels", [P, T], mybir.dt.int32, kind="ExternalInput")
    centers = nc.dram_tensor("centers", [C, D], bf16, kind="ExternalInput")
    out = nc.dram_tensor("out", [P, T + 1], f32, kind="ExternalOutput")

    with ExitStack() as ctx:
        block = ctx.enter_context(nc.Block(no_gpsimd_drain=True))
        lab = ctx.enter_context(nc.sbuf_tensor("lab", [P, T], mybir.dt.int32))
        xall = ctx.enter_context(nc.sbuf_tensor("xall", [P, T * D], bf16))
        gts = [ctx.enter_context(nc.sbuf_tensor(f"gt{t}", [P, D], bf16)) for t in range(T)]
        rowsum = ctx.enter_context(nc.sbuf_tensor("rowsum", [P, T + 1], f32))

        s_lab = ctx.enter_context(nc.semaphore("s_lab"))
        s_x = [ctx.enter_context(nc.semaphore(f"s_x{t}")) for t in range(T)]
        s_g = [ctx.enter_context(nc.semaphore(f"s_g{t}")) for t in range(T)]
        s_sub = [ctx.enter_context(nc.semaphore(f"s_sub{t}")) for t in range(T)]
        s_mul = ctx.enter_context(nc.semaphore("s_mul"))
        s_acc = ctx.enter_context(nc.semaphore("s_acc"))
        s_out = ctx.enter_context(nc.semaphore("s_out"))

        def blk(t):
            return xall[:, t * D:(t + 1) * D]

        def xdma(gpsimd, t):
            gpsimd.dma_start(out=blk(t), in_=x[:, t * D:(t + 1) * D]
                             ).then_inc(s_x[t], 16)

        def gather(gpsimd, t):
            gpsimd.indirect_dma_start(
                out=gts[t][:, :],
                out_offset=None,
                in_=centers[:, :],
                in_offset=bass.IndirectOffsetOnAxis(ap=lab[:, t:t + 1], axis=0),
            ).then_inc(s_g[t], 16)

        @block.gpsimd
        def _(gpsimd):
            # single FIFO ring, exact need-order; descriptor generation for
            # op k+1 overlaps the wire of op k
            gpsimd.dma_start(out=lab[:, :], in_=labels[:, :]).then_inc(s_lab, 16)
            xdma(gpsimd, 0)
            xdma(gpsimd, 1)
            gpsimd.wait_ge(s_lab, 16)
            gather(gpsimd, 0)
            xdma(gpsimd, 2)
            gather(gpsimd, 1)
            xdma(gpsimd, 3)
            gather(gpsimd, 2)
            gather(gpsimd, 3)

        @block.vector
        def _(vector):
            for t in range(T):
                vector.wait_ge(s_x[t], 16)
                vector.wait_ge(s_g[t], 16)
                # in-place: x block <- x - g  (bf16 keeps DVE 2x mode)
                vector.tensor_tensor(
                    out=blk(t), in0=blk(t), in1=gts[t][:, :],
                    op=mybir.AluOpType.subtract,
                ).then_inc(s_sub[t], 1)
            # second half of block 3 on DVE to shorten the tail
            vector.wait_ge(s_sub[3], 1)
            vector.tensor_tensor(
                out=gts[3][:, HD:], in0=blk(3)[:, HD:], in1=blk(3)[:, HD:],
                op=mybir.AluOpType.mult,
            ).then_inc(s_mul, 1)
            vector.wait_ge(s_mul, 1)
            vector.tensor_reduce(
                out=rowsum[:, 4:5], in_=gts[3][:, HD:],
                axis=mybir.AxisListType.X, op=mybir.AluOpType.add,
            ).then_inc(s_acc, 1)

        @block.scalar
        def _(scalar):
            for t in (0, 1, 2):
                scalar.wait_ge(s_sub[t], 1)
                scalar.activation(
                    out=blk(t), in_=blk(t),
                    func=mybir.ActivationFunctionType.Square,
                    accum_out=rowsum[:, t:t + 1],
                ).then_inc(s_acc, 1)
            scalar.wait_ge(s_sub[3], 1)
            scalar.activation(
                out=blk(3)[:, :HD], in_=blk(3)[:, :HD],
                func=mybir.ActivationFunctionType.Square,
                accum_out=rowsum[:, 3:4],
            ).then_inc(s_acc, 1)

        @block.sync
        def _(sync):
            sync.wait_ge(s_acc, 5)
            sync.dma_start(out=out[:, :], in_=rowsum[:, :]).then_inc(s_out, 16)
            if final_wait:
                sync.wait_ge(s_out, 16)

    nc.compile()
    _nc_cache[key] = nc
    return nc


def _make_in_maps(x, labels, centers):
    x = np.asarray(x, dtype=np.float32).astype(BF16)
    centers = np.ascontiguousarray(np.asarray(centers, dtype=np.float32).astype(BF16))
    lab32 = np.asarray(labels).astype(np.int32)
    in_maps = []
    for i in range(N_CORES):
        sl = slice(i * SHARD, (i + 1) * SHARD)
        in_maps.append({
            # partition p holds batch rows 4p..4p+3 of this shard
            "x": np.ascontiguousarray(x[sl]).reshape(P, T * D),
            # lab[p, k] = labels[4p + k], pairing with x column block k
            "labels": np.ascontiguousarray(lab32[sl].reshape(P, T)),
            "centers": centers,
        })
    return in_maps


def _finish(results):
    total = 0.0
    for r in results:
        rs = np.asarray(r["out"], dtype=np.float64)
        # columns 0..3: row sums for blocks 0..2 and first half of 3;
        # column 4: second half of block 3
        d = rs[:, :T].copy()
        d[:, T - 1] += rs[:, T]
        total += np.clip(d, 1e-12, 1e12).sum()
    total += (B * C - B) * 1e-12
    return np.float32(total / B)


def kernel(x, labels, centers):
    nc = _build()
    in_maps = _make_in_maps(x, labels, centers)
    res = run_bass_kernel_spmd(nc, in_maps, core_ids=list(range(N_CORES)))
    return _finish(res.results)
